# revision 21
# baseline (speedup 1.0000x reference)
"""Single-head attention (B=4, S=2048, D=1024) on 8 Trainium2 NeuronCores.

Sharding: batch x KEY-half, with a pair AllGather deduplicating the Q
projection. Core c handles batch b=c//2 and key rows [1024*h : 1024*(h+1)]
(h=c%2). Each core receives ONLY its key-half columns of x^T (xTkv) --
queries live in GLOBAL order, so the device program is rank-oblivious:

  B1own: Q^T for the core's own 1024 tokens -> bf16 shard in DRAM
         -> AllGather over pairs [[0,1],[2,3],...] -> full Q (both halves,
         global token order) -> DMA'd back into the resident SBUF tile qall
         while K/V projections run (the ~50us collective is overlapped).
  B2:    K^T = Wk^T x^T + bk (own keys) -> resident bf16 [e,k]
  B3:    V natural [k,e] -> resident bf16; bias via DVE broadcast row
  C:     per 512-query block (global order): S^T[k,q] = K^T.T Q^T ->
         exp(scale*s) on ACT writes P^T (bf16) to SBUF -> O~ = P^T.T V per
         128-query chunk -> DMA out; row-sums via DVE add-tree + one
         ones-vector matmul per block.

The HOST casts x and Wq/Wk/Wv to bf16, so every load is a plain HWDGE DMA
with 2-4KB descriptor runs: weights land as 8 d-chunk tiles [128, D] (the
matmul lhsT slices e-columns out of them), x as one [128, DC, SK] tile.
The GPSIMD queue carries only the collective trigger + readbacks, so the
gather fires the moment the last shard DMA lands.

Each core outputs UNNORMALIZED partials O~ = exp(S)V and row-sums, both in
global query order; the host combines pairs: O = (O~0+O~1)/(rs0+rs1).
(No softmax max-subtraction: scaled scores are ~N(0,1), exp never overflows.)
"""

import sys
from contextlib import ExitStack

import numpy as np
import ml_dtypes

if "/opt/trn_rl_repo" not in sys.path:
    sys.path.insert(0, "/opt/trn_rl_repo")

import concourse.bass as bass
import concourse.bacc as bacc
import concourse.tile as tile
from concourse import mybir
from concourse.bass_utils import run_bass_kernel_spmd

P = 128
S = 2048        # full sequence (queries per core, global order)
SK = 1024       # keys per core (own half)
D = 1024        # model dim
F32 = mybir.dt.float32
F32R = mybir.dt.float32r
BF16 = mybir.dt.bfloat16
NP_BF16 = ml_dtypes.bfloat16

DC = D // P     # 8 d-chunks (contraction over model dim)
EC = D // P     # 8 e-chunks (output features)
KC = SK // P    # 8 key chunks (own half)
NT = 512        # moving-operand tile (one PSUM bank of fp32)
QT = 512        # query tile for transposed scores
KSL = SK // NT  # 2 token slices of xTkv

SCALE = 1.0 / float(np.sqrt(np.float32(D)))


def build_program() -> bass.Bass:
    nc = bacc.Bacc(
        "TRN2", target_bir_lowering=False, debug=False, num_devices=8)

    xT_d = nc.dram_tensor("xTkv", [D, SK], BF16, kind="ExternalInput").ap()
    wq_d = nc.dram_tensor("Wq", [D, D], BF16, kind="ExternalInput").ap()
    wk_d = nc.dram_tensor("Wk", [D, D], BF16, kind="ExternalInput").ap()
    wv_d = nc.dram_tensor("Wv", [D, D], BF16, kind="ExternalInput").ap()
    bq_d = nc.dram_tensor("bq", [D], F32, kind="ExternalInput").ap()
    bk_d = nc.dram_tensor("bk", [D], F32, kind="ExternalInput").ap()
    bv_d = nc.dram_tensor("bv", [D], F32, kind="ExternalInput").ap()
    o_d = nc.dram_tensor("o_raw", [S, D], F32, kind="ExternalOutput").ap()
    rs_d = nc.dram_tensor("rs_raw", [S], F32, kind="ExternalOutput").ap()

    with tile.TileContext(nc) as tc, ExitStack() as ctx:
        const_p = ctx.enter_context(tc.tile_pool(name="const", bufs=1))
        io_p = ctx.enter_context(tc.tile_pool(name="io", bufs=2))
        xt_p = ctx.enter_context(tc.tile_pool(name="xt", bufs=1))
        kt_p = ctx.enter_context(tc.tile_pool(name="kt", bufs=EC))
        v_p = ctx.enter_context(tc.tile_pool(name="v", bufs=KC))
        q_p = ctx.enter_context(tc.tile_pool(name="q", bufs=1))
        wq_p = ctx.enter_context(tc.tile_pool(name="wq", bufs=DC))
        wk_p = ctx.enter_context(tc.tile_pool(name="wk", bufs=DC))
        wv_p = ctx.enter_context(tc.tile_pool(name="wv", bufs=DC))
        qsh_p = ctx.enter_context(tc.tile_pool(name="qsh", bufs=10))
        pt_p = ctx.enter_context(tc.tile_pool(name="ptp", bufs=10))
        rsum_p = ctx.enter_context(tc.tile_pool(name="rsum", bufs=5))
        st_p = ctx.enter_context(tc.tile_pool(name="stat", bufs=1))
        dram_p = ctx.enter_context(tc.tile_pool(name="cc", bufs=1, space="DRAM"))
        psB = ctx.enter_context(tc.tile_pool(name="psB", bufs=3, space="PSUM"))
        psO = ctx.enter_context(tc.tile_pool(name="psO", bufs=3, space="PSUM"))
        psA = ctx.enter_context(tc.tile_pool(name="psA", bufs=2, space="PSUM"))

        # collective bounce buffers (internal DRAM), p-major so the gather
        # output reads back with 16KB-contiguous partition runs
        q_shard = dram_p.tile([P, EC, SK], BF16)       # my Q half
        q_full = dram_p.tile([2, P, EC, SK], BF16)     # gathered pair

        # ---- sync (HWDGE) prefetch, in arrival order -------------------
        # B1own's first chain needs ALL of Wq + x slice 0.
        def wchunks(pool, src_d, nm):
            out = []
            for dcs in range(DC):
                w = pool.tile([P, D], BF16, name=nm, tag="w")
                nc.sync.dma_start(w[:], src_d[dcs * P:(dcs + 1) * P, :])
                out.append(w)
            return out

        # x slice 0 heads the queue (the sequencer dispatches one DMA per
        # ~650ns); Wq follows in two e-halves so the first chains only
        # wait for e-cols [0:512]
        xtall = xt_p.tile([P, DC, SK], BF16)
        nc.sync.dma_start(
            xtall[:, :, 0:NT],
            xT_d[:, 0:NT].rearrange("(c p) t -> p c t", p=P))
        # biases next: tiny, and the first shard ACT blocks on bqt
        bqt = const_p.tile([P, EC], F32)  # bq chunked [p, ec]
        nc.sync.dma_start(bqt[:], bq_d[:].rearrange("(c p) -> p c", p=P))
        bkt = const_p.tile([P, EC], F32)
        nc.sync.dma_start(bkt[:], bk_d[:].rearrange("(c p) -> p c", p=P))
        wq_c = []
        for dcs in range(DC):
            w = wq_p.tile([P, D], BF16, name="wqc", tag="w")
            nc.sync.dma_start(
                w[:, 0:D // 2], wq_d[dcs * P:(dcs + 1) * P, 0:D // 2])
            wq_c.append(w)
        for dcs in range(DC):
            nc.sync.dma_start(
                wq_c[dcs][:, D // 2:D], wq_d[dcs * P:(dcs + 1) * P, D // 2:D])
        nc.sync.dma_start(
            xtall[:, :, NT:SK],
            xT_d[:, NT:SK].rearrange("(c p) t -> p c t", p=P))

        # constants (DVE memsets + casts)
        ones_raw = const_p.tile([P, 1], F32)
        nc.vector.memset(ones_raw[:], 1.0)
        ones = const_p.tile([P, 1], F32R)  # column of ones: lhsT for row-sums
        nc.vector.tensor_copy(ones[:], ones_raw[:])
        onesr_raw = const_p.tile([1, P], F32)
        nc.vector.memset(onesr_raw[:], 1.0)
        ones_row = const_p.tile([1, P], F32R)  # row of ones: bv broadcast mm
        nc.vector.tensor_copy(ones_row[:], onesr_raw[:])
        # bv as a single f32r row (tiny SWDGE casting DMA, heads the
        # gpsimd queue ahead of the collective trigger)
        bvr = const_p.tile([1, D], F32R)
        nc.gpsimd.dma_start(bvr[:], bv_d[:].rearrange("(o d) -> o d", o=1))

        # ---- Phase B1own: Q^T for MY tokens -> DRAM shard --------------
        # kt_i outer so the shard fills token-slice by token-slice; the
        # wk/wv chunk loads are interleaved BEHIND each slice's shard-out
        # DMAs in the sync FIFO so the collective trigger fires early
        wk_c = wv_c = None

        def b1own_slice(kt_i):
            for ec in range(EC):
                ps = psB.tile([P, NT], F32)
                for dc in range(DC):
                    nc.tensor.matmul(
                        ps[:],
                        (wq_c[dc][:, ec * P:(ec + 1) * P]),
                        (xtall[:, dc, kt_i * NT:(kt_i + 1) * NT]),
                        start=(dc == 0), stop=(dc == DC - 1),
                    )
                qs = qsh_p.tile([P, NT], BF16, name="qs", tag="qsh")
                nc.scalar.activation(
                    qs[:], ps[:],
                    mybir.ActivationFunctionType.Identity,
                    bias=bqt[:, ec:ec + 1],
                )
                nc.sync.dma_start(
                    q_shard[:, ec, kt_i * NT:(kt_i + 1) * NT], qs[:])

        b1own_slice(0)
        wk_c = wchunks(wk_p, wk_d, "wkc")
        b1own_slice(1)
        wv_c = wchunks(wv_p, wv_d, "wvc")

        # ---- pair AllGather of the Q halves (overlaps B2/B3) -----------
        # the gpsimd FIFO holds only [bvr, trigger, readbacks], so the
        # trigger dispatches the moment the shard DMAs land (~45us).
        nc.gpsimd.collective_compute(
            "AllGather",
            mybir.AluOpType.bypass,
            replica_groups=[[0, 1], [2, 3], [4, 5], [6, 7]],
            ins=[q_shard.opt()],
            outs=[q_full.opt()],
        )
        # readback: slot r holds GLOBAL tokens r*1024:(r+1)*1024; slot 0
        # first (phase C consumes query blocks in global order)
        qall = q_p.tile([P, EC, S], BF16)
        for r in range(2):
            nc.gpsimd.dma_start(
                qall[:, :, r * SK:(r + 1) * SK], q_full[r, :, :, :])

        # ---- Phase B2: K^T (own keys) resident bf16 --------------------
        kt = [kt_p.tile([P, SK], BF16, name=f"kt{ec}", tag="kt")
              for ec in range(EC)]
        for ec in range(EC):
            for kt_i in range(KSL):
                ps = psB.tile([P, NT], F32)
                for dc in range(DC):
                    nc.tensor.matmul(
                        ps[:],
                        (wk_c[dc][:, ec * P:(ec + 1) * P]),
                        (xtall[:, dc, kt_i * NT:(kt_i + 1) * NT]),
                        start=(dc == 0), stop=(dc == DC - 1),
                    )
                nc.scalar.activation(
                    kt[ec][:, kt_i * NT:(kt_i + 1) * NT], ps[:],
                    mybir.ActivationFunctionType.Identity,
                    bias=bkt[:, ec:ec + 1],
                )

        # bv broadcast row -> [128, D] f32 (rank-1 ones x bv matmul)
        bv_rep = const_p.tile([P, D], F32)
        for et in range(D // NT):
            ps = psB.tile([P, NT], F32)
            nc.tensor.matmul(
                ps[:],
                (ones_row[0:1, :]),
                (bvr[0:1, et * NT:(et + 1) * NT]),
                start=True, stop=True,
            )
            nc.scalar.copy(bv_rep[:, et * NT:(et + 1) * NT], ps[:])

        # ---- Phase B3: V natural [k, e] (own keys) resident bf16 -------
        v = [v_p.tile([P, D], BF16, name=f"v{kc}", tag="v")
             for kc in range(KC)]
        for et in range(D // NT):
            for kc in range(KC):
                ps = psB.tile([P, NT], F32)
                for dc in range(DC):
                    nc.tensor.matmul(
                        ps[:],
                        (xtall[:, dc, kc * P:(kc + 1) * P]),
                        (wv_c[dc][:, et * NT:(et + 1) * NT]),
                        start=(dc == 0), stop=(dc == DC - 1),
                    )
                nc.vector.tensor_add(
                    v[kc][:, et * NT:(et + 1) * NT], ps[:],
                    bv_rep[:, et * NT:(et + 1) * NT])

        # ---- Phase C: attention, transposed scores ---------------------
        for qq in range(S // QT):
            # S^T[k, q] per key chunk; exp writes P^T (bf16) straight to SBUF
            ptt = [pt_p.tile([P, QT], BF16, tag="ptp", name=f"ptt{kc}")
                   for kc in range(KC)]
            for kc in range(KC):
                ps = psB.tile([P, QT], F32)
                for ec in range(EC):
                    nc.tensor.matmul(
                        ps[:],
                        (kt[ec][:, kc * P:(kc + 1) * P]),
                        (qall[:, ec, qq * QT:(qq + 1) * QT]),
                        start=(ec == 0), stop=(ec == EC - 1),
                    )
                nc.scalar.activation(
                    ptt[kc][:], ps[:],
                    mybir.ActivationFunctionType.Exp,
                    scale=SCALE,
                )

            # O~ = P^T.T @ V, per 128-query chunk. Paces slower than the
            # ACT exp stream, absorbing its latency.
            for qc in range(QT // P):
                o_sb = io_p.tile([P, D], F32, name="osb", tag="io")
                for et in range(D // NT):
                    ps = psO.tile([P, NT], F32, name="pso")
                    for kc in range(KC):
                        nc.tensor.matmul(
                            ps[:],
                            (ptt[kc][:, qc * P:(qc + 1) * P]),
                            (v[kc][:, et * NT:(et + 1) * NT]),
                            start=(kc == 0), stop=(kc == KC - 1),
                        )
                    nc.vector.tensor_copy(
                        o_sb[:, et * NT:(et + 1) * NT], ps[:])
                row0 = qq * QT + qc * P
                nc.sync.dma_start(o_d[row0:row0 + P, :], o_sb[:])

            # partial row-sums: DVE add-tree over the 8 P^T tiles, then a
            # single ones-vector matmul (f32r, 1 col/cycle).
            def _radd(a, b):
                t = rsum_p.tile([P, QT], F32R, name="racc", tag="racc")
                nc.vector.tensor_add(t[:], a, b)
                return t
            s01 = _radd(ptt[0][:], ptt[1][:])
            s23 = _radd(ptt[2][:], ptt[3][:])
            s45 = _radd(ptt[4][:], ptt[5][:])
            s67 = _radd(ptt[6][:], ptt[7][:])
            s03 = _radd(s01[:], s23[:])
            s47 = _radd(s45[:], s67[:])
            tot = _radd(s03[:], s47[:])
            ps_rs = psA.tile([1, QT], F32, name="ps_rs")
            nc.tensor.matmul(
                ps_rs[:], (ones[:, 0:1]), (tot[:]), start=True, stop=True)
            rs_sb = st_p.tile([1, QT], F32, name="rs_sb", tag="rs")
            nc.vector.tensor_copy(rs_sb[:], ps_rs[:])
            nc.sync.dma_start(
                rs_d[qq * QT:(qq + 1) * QT].rearrange("(o q) -> o q", o=1),
                rs_sb[:])

    nc.compile()
    return nc


_CACHE: dict = {}


def _get_program() -> bass.Bass:
    if "nc" not in _CACHE:
        _CACHE["nc"] = build_program()
    return _CACHE["nc"]


def kernel(x, Wq, bq, Wk, bk, Wv, bv, _trace=False, _trace_kwargs=None):
    nc = _get_program()
    x = np.asarray(x, dtype=np.float32)
    shared = {
        "Wq": np.ascontiguousarray(np.asarray(Wq, np.float32).astype(NP_BF16)),
        "bq": np.ascontiguousarray(np.asarray(bq, np.float32)),
        "Wk": np.ascontiguousarray(np.asarray(Wk, np.float32).astype(NP_BF16)),
        "bk": np.ascontiguousarray(np.asarray(bk, np.float32)),
        "Wv": np.ascontiguousarray(np.asarray(Wv, np.float32).astype(NP_BF16)),
        "bv": np.ascontiguousarray(np.asarray(bv, np.float32)),
    }
    in_maps = []
    for c in range(8):
        b, h = divmod(c, 2)
        xkv = x[b].T[:, h * SK:(h + 1) * SK]  # my key-half columns of x^T
        in_maps.append(
            {"xTkv": np.ascontiguousarray(xkv.astype(NP_BF16)), **shared})

    res = run_bass_kernel_spmd(
        nc, in_maps, list(range(8)),
        trace=_trace, **(_trace_kwargs or {}),
    )
    out = np.empty((4, S, D), dtype=np.float32)
    for b in range(4):
        o0 = res.results[2 * b]["o_raw"].astype(np.float64)
        r0 = res.results[2 * b]["rs_raw"].astype(np.float64)
        o1 = res.results[2 * b + 1]["o_raw"].astype(np.float64)
        r1 = res.results[2 * b + 1]["rs_raw"].astype(np.float64)
        out[b] = ((o0 + o1) / (r0 + r1)[:, None]).astype(np.float32)
    if _trace:
        return out, res
    return out


# revision 22
# speedup vs baseline: 1.1622x; 1.1622x over previous
"""Single-head attention (B=4, S=2048, D=1024) on 8 Trainium2 NeuronCores.

Sharding: batch x KEY-half, with a pair AllGather deduplicating the Q
projection. Core c handles batch b=c//2 and key rows [1024*h : 1024*(h+1)]
(h=c%2). Each core receives ONLY its key-half columns of x^T (xTkv) --
queries live in GLOBAL order, so the device program is rank-oblivious:

  B1own: Q^T for the core's own 1024 tokens -> bf16 shard in DRAM
         -> AllGather over pairs [[0,1],[2,3],...] -> full Q (both halves,
         global token order) -> DMA'd back into the resident SBUF tile qall
         while K/V projections run (the ~50us collective is overlapped).
  B2:    K^T = Wk^T x^T + bk (own keys) -> resident bf16 [e,k]
  B3:    V natural [k,e] -> resident bf16; bias via DVE broadcast row
  C:     per 512-query block (global order): S^T[k,q] = K^T.T Q^T ->
         exp(scale*s) on ACT writes P^T (bf16) to SBUF -> O~ = P^T.T V per
         128-query chunk -> DMA out; row-sums via DVE add-tree + one
         ones-vector matmul per block.

The HOST casts x and Wq/Wk/Wv to bf16, so every load is a plain HWDGE DMA
with 2-4KB descriptor runs: weights land as 8 d-chunk tiles [128, D] (the
matmul lhsT slices e-columns out of them), x as one [128, DC, SK] tile.
The GPSIMD queue carries only the collective trigger + readbacks, so the
gather fires the moment the last shard DMA lands.

Each core outputs UNNORMALIZED partials O~ = exp(S)V and row-sums, both in
global query order; the host combines pairs: O = (O~0+O~1)/(rs0+rs1).
(No softmax max-subtraction: scaled scores are ~N(0,1), exp never overflows.)
"""

import sys
from contextlib import ExitStack

import numpy as np
import ml_dtypes

if "/opt/trn_rl_repo" not in sys.path:
    sys.path.insert(0, "/opt/trn_rl_repo")

import concourse.bass as bass
import concourse.bacc as bacc
import concourse.tile as tile
from concourse import mybir
from concourse.bass_utils import run_bass_kernel_spmd

P = 128
S = 2048        # full sequence (queries per core, global order)
SK = 1024       # keys per core (own half)
D = 1024        # model dim
F32 = mybir.dt.float32
F32R = mybir.dt.float32r
BF16 = mybir.dt.bfloat16
NP_BF16 = ml_dtypes.bfloat16

DC = D // P     # 8 d-chunks (contraction over model dim)
EC = D // P     # 8 e-chunks (output features)
KC = SK // P    # 8 key chunks (own half)
NT = 512        # moving-operand tile (one PSUM bank of fp32)
QT = 512        # query tile for transposed scores
KSL = SK // NT  # 2 token slices of xTkv

SCALE = 1.0 / float(np.sqrt(np.float32(D)))


def build_program() -> bass.Bass:
    nc = bacc.Bacc(
        "TRN2", target_bir_lowering=False, debug=False, num_devices=8)

    xT_d = nc.dram_tensor("xTkv", [D, SK], BF16, kind="ExternalInput").ap()
    wq_d = nc.dram_tensor("Wq", [D, D], BF16, kind="ExternalInput").ap()
    wk_d = nc.dram_tensor("Wk", [D, D], BF16, kind="ExternalInput").ap()
    wv_d = nc.dram_tensor("Wv", [D, D], BF16, kind="ExternalInput").ap()
    bq_d = nc.dram_tensor("bq", [D], F32, kind="ExternalInput").ap()
    bk_d = nc.dram_tensor("bk", [D], F32, kind="ExternalInput").ap()
    bv_d = nc.dram_tensor("bv", [D], F32, kind="ExternalInput").ap()
    o_d = nc.dram_tensor("o_raw", [S, D], F32, kind="ExternalOutput").ap()
    rs_d = nc.dram_tensor("rs_raw", [S], F32, kind="ExternalOutput").ap()

    with tile.TileContext(nc) as tc, ExitStack() as ctx:
        const_p = ctx.enter_context(tc.tile_pool(name="const", bufs=1))
        io_p = ctx.enter_context(tc.tile_pool(name="io", bufs=2))
        xt_p = ctx.enter_context(tc.tile_pool(name="xt", bufs=1))
        kt_p = ctx.enter_context(tc.tile_pool(name="kt", bufs=EC))
        v_p = ctx.enter_context(tc.tile_pool(name="v", bufs=KC))
        q_p = ctx.enter_context(tc.tile_pool(name="q", bufs=1))
        wq_p = ctx.enter_context(tc.tile_pool(name="wq", bufs=DC))
        wk_p = ctx.enter_context(tc.tile_pool(name="wk", bufs=DC))
        wv_p = ctx.enter_context(tc.tile_pool(name="wv", bufs=DC))
        qsh_p = ctx.enter_context(tc.tile_pool(name="qsh", bufs=10))
        pt_p = ctx.enter_context(tc.tile_pool(name="ptp", bufs=10))
        rsum_p = ctx.enter_context(tc.tile_pool(name="rsum", bufs=5))
        st_p = ctx.enter_context(tc.tile_pool(name="stat", bufs=1))
        dram_p = ctx.enter_context(tc.tile_pool(name="cc", bufs=1, space="DRAM"))
        psB = ctx.enter_context(tc.tile_pool(name="psB", bufs=3, space="PSUM"))
        psO = ctx.enter_context(tc.tile_pool(name="psO", bufs=3, space="PSUM"))
        psA = ctx.enter_context(tc.tile_pool(name="psA", bufs=2, space="PSUM"))

        # collective bounce buffers (internal DRAM), p-major so the gather
        # output reads back with 16KB-contiguous partition runs
        q_shard = dram_p.tile([P, EC, SK], BF16)       # my Q half
        q_full = dram_p.tile([2, P, EC, SK], BF16)     # gathered pair

        # ---- sync (HWDGE) prefetch, in arrival order -------------------
        # B1own's first chain needs ALL of Wq + x slice 0.
        def wchunks(pool, src_d, nm):
            out = []
            for dcs in range(DC):
                w = pool.tile([P, D], BF16, name=nm, tag="w")
                nc.sync.dma_start(w[:], src_d[dcs * P:(dcs + 1) * P, :])
                out.append(w)
            return out

        # x slice 0 heads the queue (the sequencer dispatches one DMA per
        # ~650ns); Wq follows in two e-halves so the first chains only
        # wait for e-cols [0:512]
        xtall = xt_p.tile([P, DC, SK], BF16)
        nc.sync.dma_start(
            xtall[:, :, 0:NT],
            xT_d[:, 0:NT].rearrange("(c p) t -> p c t", p=P))
        wq_c = []
        for dcs in range(DC):
            w = wq_p.tile([P, D], BF16, name="wqc", tag="w")
            nc.sync.dma_start(
                w[:, 0:D // 2], wq_d[dcs * P:(dcs + 1) * P, 0:D // 2])
            wq_c.append(w)
        for dcs in range(DC):
            nc.sync.dma_start(
                wq_c[dcs][:, D // 2:D], wq_d[dcs * P:(dcs + 1) * P, D // 2:D])
        bqt = const_p.tile([P, EC], F32)  # bq chunked [p, ec]
        nc.sync.dma_start(bqt[:], bq_d[:].rearrange("(c p) -> p c", p=P))
        bkt = const_p.tile([P, EC], F32)
        nc.sync.dma_start(bkt[:], bk_d[:].rearrange("(c p) -> p c", p=P))
        nc.sync.dma_start(
            xtall[:, :, NT:SK],
            xT_d[:, NT:SK].rearrange("(c p) t -> p c t", p=P))

        # constants (DVE memsets + casts)
        ones_raw = const_p.tile([P, 1], F32)
        nc.vector.memset(ones_raw[:], 1.0)
        ones = const_p.tile([P, 1], F32R)  # column of ones: lhsT for row-sums
        nc.vector.tensor_copy(ones[:], ones_raw[:])
        onesr_raw = const_p.tile([1, P], F32)
        nc.vector.memset(onesr_raw[:], 1.0)
        ones_row = const_p.tile([1, P], F32R)  # row of ones: bv broadcast mm
        nc.vector.tensor_copy(ones_row[:], onesr_raw[:])
        # bv as a single f32r row (tiny SWDGE casting DMA, heads the
        # gpsimd queue ahead of the collective trigger)
        bvr = const_p.tile([1, D], F32R)
        nc.gpsimd.dma_start(bvr[:], bv_d[:].rearrange("(o d) -> o d", o=1))

        # ---- Phase B1own: Q^T for MY tokens -> DRAM shard --------------
        # kt_i outer so the shard fills token-slice by token-slice; the
        # wk/wv chunk loads are interleaved BEHIND each slice's shard-out
        # DMAs in the sync FIFO so the collective trigger fires early
        wk_c = wv_c = None

        def b1own_slice(kt_i):
            for ec in range(EC):
                ps = psB.tile([P, NT], F32)
                for dc in range(DC):
                    nc.tensor.matmul(
                        ps[:],
                        (wq_c[dc][:, ec * P:(ec + 1) * P]),
                        (xtall[:, dc, kt_i * NT:(kt_i + 1) * NT]),
                        start=(dc == 0), stop=(dc == DC - 1),
                    )
                qs = qsh_p.tile([P, NT], BF16, name="qs", tag="qsh")
                nc.scalar.activation(
                    qs[:], ps[:],
                    mybir.ActivationFunctionType.Identity,
                    bias=bqt[:, ec:ec + 1],
                )
                nc.sync.dma_start(
                    q_shard[:, ec, kt_i * NT:(kt_i + 1) * NT], qs[:])

        b1own_slice(0)
        wk_c = wchunks(wk_p, wk_d, "wkc")
        b1own_slice(1)
        wv_c = wchunks(wv_p, wv_d, "wvc")

        # ---- pair AllGather of the Q halves (overlaps B2/B3) -----------
        # the gpsimd FIFO holds only [bvr, trigger, readbacks], so the
        # trigger dispatches the moment the shard DMAs land (~45us).
        nc.gpsimd.collective_compute(
            "AllGather",
            mybir.AluOpType.bypass,
            replica_groups=[[0, 1], [2, 3], [4, 5], [6, 7]],
            ins=[q_shard.opt()],
            outs=[q_full.opt()],
        )
        # readback: slot r holds GLOBAL tokens r*1024:(r+1)*1024; slot 0
        # first (phase C consumes query blocks in global order)
        qall = q_p.tile([P, EC, S], BF16)
        for r in range(2):
            nc.gpsimd.dma_start(
                qall[:, :, r * SK:(r + 1) * SK], q_full[r, :, :, :])

        # ---- Phase B2: K^T (own keys) resident bf16 --------------------
        kt = [kt_p.tile([P, SK], BF16, name=f"kt{ec}", tag="kt")
              for ec in range(EC)]
        for ec in range(EC):
            for kt_i in range(KSL):
                ps = psB.tile([P, NT], F32)
                for dc in range(DC):
                    nc.tensor.matmul(
                        ps[:],
                        (wk_c[dc][:, ec * P:(ec + 1) * P]),
                        (xtall[:, dc, kt_i * NT:(kt_i + 1) * NT]),
                        start=(dc == 0), stop=(dc == DC - 1),
                    )
                nc.scalar.activation(
                    kt[ec][:, kt_i * NT:(kt_i + 1) * NT], ps[:],
                    mybir.ActivationFunctionType.Identity,
                    bias=bkt[:, ec:ec + 1],
                )

        # bv broadcast row -> [128, D] f32 (rank-1 ones x bv matmul)
        bv_rep = const_p.tile([P, D], F32)
        for et in range(D // NT):
            ps = psB.tile([P, NT], F32)
            nc.tensor.matmul(
                ps[:],
                (ones_row[0:1, :]),
                (bvr[0:1, et * NT:(et + 1) * NT]),
                start=True, stop=True,
            )
            nc.scalar.copy(bv_rep[:, et * NT:(et + 1) * NT], ps[:])

        # ---- Phase B3: V natural [k, e] (own keys) resident bf16 -------
        v = [v_p.tile([P, D], BF16, name=f"v{kc}", tag="v")
             for kc in range(KC)]
        for et in range(D // NT):
            for kc in range(KC):
                ps = psB.tile([P, NT], F32)
                for dc in range(DC):
                    nc.tensor.matmul(
                        ps[:],
                        (xtall[:, dc, kc * P:(kc + 1) * P]),
                        (wv_c[dc][:, et * NT:(et + 1) * NT]),
                        start=(dc == 0), stop=(dc == DC - 1),
                    )
                nc.vector.tensor_add(
                    v[kc][:, et * NT:(et + 1) * NT], ps[:],
                    bv_rep[:, et * NT:(et + 1) * NT])

        # ---- Phase C: attention, transposed scores ---------------------
        for qq in range(S // QT):
            # S^T[k, q] per key chunk; exp writes P^T (bf16) straight to SBUF
            ptt = [pt_p.tile([P, QT], BF16, tag="ptp", name=f"ptt{kc}")
                   for kc in range(KC)]
            for kc in range(KC):
                ps = psB.tile([P, QT], F32)
                for ec in range(EC):
                    nc.tensor.matmul(
                        ps[:],
                        (kt[ec][:, kc * P:(kc + 1) * P]),
                        (qall[:, ec, qq * QT:(qq + 1) * QT]),
                        start=(ec == 0), stop=(ec == EC - 1),
                    )
                nc.scalar.activation(
                    ptt[kc][:], ps[:],
                    mybir.ActivationFunctionType.Exp,
                    scale=SCALE,
                )

            # O~ = P^T.T @ V, per 128-query chunk. Paces slower than the
            # ACT exp stream, absorbing its latency.
            for qc in range(QT // P):
                o_sb = io_p.tile([P, D], F32, name="osb", tag="io")
                for et in range(D // NT):
                    ps = psO.tile([P, NT], F32, name="pso")
                    for kc in range(KC):
                        nc.tensor.matmul(
                            ps[:],
                            (ptt[kc][:, qc * P:(qc + 1) * P]),
                            (v[kc][:, et * NT:(et + 1) * NT]),
                            start=(kc == 0), stop=(kc == KC - 1),
                        )
                    nc.vector.tensor_copy(
                        o_sb[:, et * NT:(et + 1) * NT], ps[:])
                row0 = qq * QT + qc * P
                nc.sync.dma_start(o_d[row0:row0 + P, :], o_sb[:])

            # partial row-sums: DVE add-tree over the 8 P^T tiles, then a
            # single ones-vector matmul (f32r, 1 col/cycle).
            def _radd(a, b):
                t = rsum_p.tile([P, QT], F32R, name="racc", tag="racc")
                nc.vector.tensor_add(t[:], a, b)
                return t
            s01 = _radd(ptt[0][:], ptt[1][:])
            s23 = _radd(ptt[2][:], ptt[3][:])
            s45 = _radd(ptt[4][:], ptt[5][:])
            s67 = _radd(ptt[6][:], ptt[7][:])
            s03 = _radd(s01[:], s23[:])
            s47 = _radd(s45[:], s67[:])
            tot = _radd(s03[:], s47[:])
            ps_rs = psA.tile([1, QT], F32, name="ps_rs")
            nc.tensor.matmul(
                ps_rs[:], (ones[:, 0:1]), (tot[:]), start=True, stop=True)
            rs_sb = st_p.tile([1, QT], F32, name="rs_sb", tag="rs")
            nc.vector.tensor_copy(rs_sb[:], ps_rs[:])
            nc.sync.dma_start(
                rs_d[qq * QT:(qq + 1) * QT].rearrange("(o q) -> o q", o=1),
                rs_sb[:])

    nc.compile()
    return nc


_CACHE: dict = {}


def _get_program() -> bass.Bass:
    if "nc" not in _CACHE:
        _CACHE["nc"] = build_program()
    return _CACHE["nc"]


def kernel(x, Wq, bq, Wk, bk, Wv, bv, _trace=False, _trace_kwargs=None):
    nc = _get_program()
    x = np.asarray(x, dtype=np.float32)
    shared = {
        "Wq": np.ascontiguousarray(np.asarray(Wq, np.float32).astype(NP_BF16)),
        "bq": np.ascontiguousarray(np.asarray(bq, np.float32)),
        "Wk": np.ascontiguousarray(np.asarray(Wk, np.float32).astype(NP_BF16)),
        "bk": np.ascontiguousarray(np.asarray(bk, np.float32)),
        "Wv": np.ascontiguousarray(np.asarray(Wv, np.float32).astype(NP_BF16)),
        "bv": np.ascontiguousarray(np.asarray(bv, np.float32)),
    }
    in_maps = []
    for c in range(8):
        b, h = divmod(c, 2)
        xkv = x[b].T[:, h * SK:(h + 1) * SK]  # my key-half columns of x^T
        in_maps.append(
            {"xTkv": np.ascontiguousarray(xkv.astype(NP_BF16)), **shared})

    res = run_bass_kernel_spmd(
        nc, in_maps, list(range(8)),
        trace=_trace, **(_trace_kwargs or {}),
    )
    out = np.empty((4, S, D), dtype=np.float32)
    for b in range(4):
        o0 = res.results[2 * b]["o_raw"].astype(np.float64)
        r0 = res.results[2 * b]["rs_raw"].astype(np.float64)
        o1 = res.results[2 * b + 1]["o_raw"].astype(np.float64)
        r1 = res.results[2 * b + 1]["rs_raw"].astype(np.float64)
        out[b] = ((o0 + o1) / (r0 + r1)[:, None]).astype(np.float32)
    if _trace:
        return out, res
    return out


# revision 23
# speedup vs baseline: 1.1913x; 1.0250x over previous
"""Single-head attention (B=4, S=2048, D=1024) on 8 Trainium2 NeuronCores.

Sharding: batch x KEY-half, with a pair AllGather deduplicating the Q
projection. Core c handles batch b=c//2 and key rows [1024*h : 1024*(h+1)]
(h=c%2). Each core receives ONLY its key-half columns of x^T (xTkv) --
queries live in GLOBAL order, so the device program is rank-oblivious:

  B1own: Q^T for the core's own 1024 tokens -> bf16 shard in DRAM
         -> AllGather over pairs [[0,1],[2,3],...] -> full Q (both halves,
         global token order) -> DMA'd back into the resident SBUF tile qall
         while K/V projections run (the ~50us collective is overlapped).
  B2:    K^T = Wk^T x^T + bk (own keys) -> resident bf16 [e,k]
  B3:    V natural [k,e] -> resident bf16; bias via DVE broadcast row
  C:     per 512-query block (global order): S^T[k,q] = K^T.T Q^T ->
         exp(scale*s) on ACT writes P^T (bf16) to SBUF -> O~ = P^T.T V per
         128-query chunk -> DMA out; row-sums via DVE add-tree + one
         ones-vector matmul per block.

The HOST casts x and Wq/Wk/Wv to bf16, so every load is a plain HWDGE DMA
with 2-4KB descriptor runs: weights land as 8 d-chunk tiles [128, D] (the
matmul lhsT slices e-columns out of them), x as one [128, DC, SK] tile.
The GPSIMD queue carries only the collective trigger + readbacks, so the
gather fires the moment the last shard DMA lands.

Each core outputs UNNORMALIZED partials O~ = exp(S)V and row-sums, both in
global query order; the host combines pairs: O = (O~0+O~1)/(rs0+rs1).
(No softmax max-subtraction: scaled scores are ~N(0,1), exp never overflows.)
"""

import sys
from contextlib import ExitStack

import numpy as np
import ml_dtypes

if "/opt/trn_rl_repo" not in sys.path:
    sys.path.insert(0, "/opt/trn_rl_repo")

import concourse.bass as bass
import concourse.bacc as bacc
import concourse.tile as tile
from concourse import mybir
from concourse.bass_utils import run_bass_kernel_spmd

P = 128
S = 2048        # full sequence (queries per core, global order)
SK = 1024       # keys per core (own half)
D = 1024        # model dim
F32 = mybir.dt.float32
F32R = mybir.dt.float32r
BF16 = mybir.dt.bfloat16
NP_BF16 = ml_dtypes.bfloat16

DC = D // P     # 8 d-chunks (contraction over model dim)
EC = D // P     # 8 e-chunks (output features)
KC = SK // P    # 8 key chunks (own half)
NT = 512        # moving-operand tile (one PSUM bank of fp32)
QT = 512        # query tile for transposed scores
KSL = SK // NT  # 2 token slices of xTkv

SCALE = 1.0 / float(np.sqrt(np.float32(D)))


def build_program() -> bass.Bass:
    nc = bacc.Bacc(
        "TRN2", target_bir_lowering=False, debug=False, num_devices=8)

    xT_d = nc.dram_tensor("xTkv", [D, SK], BF16, kind="ExternalInput").ap()
    wq_d = nc.dram_tensor("Wq", [D, D], BF16, kind="ExternalInput").ap()
    wk_d = nc.dram_tensor("Wk", [D, D], BF16, kind="ExternalInput").ap()
    wv_d = nc.dram_tensor("Wv", [D, D], BF16, kind="ExternalInput").ap()
    bq_d = nc.dram_tensor("bq", [D], F32, kind="ExternalInput").ap()
    bk_d = nc.dram_tensor("bk", [D], F32, kind="ExternalInput").ap()
    bv_d = nc.dram_tensor("bv", [D], F32, kind="ExternalInput").ap()
    o_d = nc.dram_tensor("o_raw", [S, D], F32, kind="ExternalOutput").ap()
    rs_d = nc.dram_tensor("rs_raw", [S], F32, kind="ExternalOutput").ap()

    with tile.TileContext(nc) as tc, ExitStack() as ctx:
        const_p = ctx.enter_context(tc.tile_pool(name="const", bufs=1))
        io_p = ctx.enter_context(tc.tile_pool(name="io", bufs=2))
        xt_p = ctx.enter_context(tc.tile_pool(name="xt", bufs=1))
        kt_p = ctx.enter_context(tc.tile_pool(name="kt", bufs=EC))
        v_p = ctx.enter_context(tc.tile_pool(name="v", bufs=KC))
        q_p = ctx.enter_context(tc.tile_pool(name="q", bufs=1))
        wq_p = ctx.enter_context(tc.tile_pool(name="wq", bufs=DC))
        wk_p = ctx.enter_context(tc.tile_pool(name="wk", bufs=DC))
        wv_p = ctx.enter_context(tc.tile_pool(name="wv", bufs=DC))
        qsh_p = ctx.enter_context(tc.tile_pool(name="qsh", bufs=10))
        pt_p = ctx.enter_context(tc.tile_pool(name="ptp", bufs=10))
        rsum_p = ctx.enter_context(tc.tile_pool(name="rsum", bufs=5))
        st_p = ctx.enter_context(tc.tile_pool(name="stat", bufs=1))
        dram_p = ctx.enter_context(tc.tile_pool(name="cc", bufs=1, space="DRAM"))
        # PSUM bank split: psB gets 5 so the projection chains can run
        # far ahead of their (bias-gated) ACT drains; the PV pipeline and
        # the once-per-block rowsum need only 2 and 1.
        psB = ctx.enter_context(tc.tile_pool(name="psB", bufs=5, space="PSUM"))
        psO = ctx.enter_context(tc.tile_pool(name="psO", bufs=2, space="PSUM"))
        psA = ctx.enter_context(tc.tile_pool(name="psA", bufs=1, space="PSUM"))

        # collective bounce buffers (internal DRAM), p-major so the gather
        # output reads back with 16KB-contiguous partition runs
        q_shard = dram_p.tile([P, EC, SK], BF16)       # my Q half
        q_full = dram_p.tile([2, P, EC, SK], BF16)     # gathered pair

        # ---- sync (HWDGE) prefetch, in arrival order -------------------
        # B1own's first chain needs ALL of Wq + x slice 0.
        def wchunks(pool, src_d, nm):
            out = []
            for dcs in range(DC):
                w = pool.tile([P, D], BF16, name=nm, tag="w")
                nc.sync.dma_start(w[:], src_d[dcs * P:(dcs + 1) * P, :])
                out.append(w)
            return out

        # x slice 0 heads the queue (the sequencer dispatches one DMA per
        # ~650ns); Wq follows in two e-halves so the first chains only
        # wait for e-cols [0:512]
        xtall = xt_p.tile([P, DC, SK], BF16)
        nc.sync.dma_start(
            xtall[:, :, 0:NT],
            xT_d[:, 0:NT].rearrange("(c p) t -> p c t", p=P))
        wq_c = []
        for dcs in range(DC):
            w = wq_p.tile([P, D], BF16, name="wqc", tag="w")
            nc.sync.dma_start(
                w[:, 0:D // 2], wq_d[dcs * P:(dcs + 1) * P, 0:D // 2])
            wq_c.append(w)
        for dcs in range(DC):
            nc.sync.dma_start(
                wq_c[dcs][:, D // 2:D], wq_d[dcs * P:(dcs + 1) * P, D // 2:D])
        bqt = const_p.tile([P, EC], F32)  # bq chunked [p, ec]
        nc.sync.dma_start(bqt[:], bq_d[:].rearrange("(c p) -> p c", p=P))
        bkt = const_p.tile([P, EC], F32)
        nc.sync.dma_start(bkt[:], bk_d[:].rearrange("(c p) -> p c", p=P))
        nc.sync.dma_start(
            xtall[:, :, NT:SK],
            xT_d[:, NT:SK].rearrange("(c p) t -> p c t", p=P))

        # constants (DVE memsets + casts)
        ones_raw = const_p.tile([P, 1], F32)
        nc.vector.memset(ones_raw[:], 1.0)
        ones = const_p.tile([P, 1], F32R)  # column of ones: lhsT for row-sums
        nc.vector.tensor_copy(ones[:], ones_raw[:])
        onesr_raw = const_p.tile([1, P], F32)
        nc.vector.memset(onesr_raw[:], 1.0)
        ones_row = const_p.tile([1, P], F32R)  # row of ones: bv broadcast mm
        nc.vector.tensor_copy(ones_row[:], onesr_raw[:])
        # bv as a single f32r row (tiny SWDGE casting DMA, heads the
        # gpsimd queue ahead of the collective trigger)
        bvr = const_p.tile([1, D], F32R)
        nc.gpsimd.dma_start(bvr[:], bv_d[:].rearrange("(o d) -> o d", o=1))

        # ---- Phase B1own: Q^T for MY tokens -> DRAM shard --------------
        # kt_i outer so the shard fills token-slice by token-slice; the
        # wk/wv chunk loads are interleaved BEHIND each slice's shard-out
        # DMAs in the sync FIFO so the collective trigger fires early
        wk_c = wv_c = None

        def b1own_slice(kt_i):
            for ec in range(EC):
                ps = psB.tile([P, NT], F32)
                for dc in range(DC):
                    nc.tensor.matmul(
                        ps[:],
                        (wq_c[dc][:, ec * P:(ec + 1) * P]),
                        (xtall[:, dc, kt_i * NT:(kt_i + 1) * NT]),
                        start=(dc == 0), stop=(dc == DC - 1),
                    )
                qs = qsh_p.tile([P, NT], BF16, name="qs", tag="qsh")
                nc.scalar.activation(
                    qs[:], ps[:],
                    mybir.ActivationFunctionType.Identity,
                    bias=bqt[:, ec:ec + 1],
                )
                nc.sync.dma_start(
                    q_shard[:, ec, kt_i * NT:(kt_i + 1) * NT], qs[:])

        b1own_slice(0)
        wk_c = wchunks(wk_p, wk_d, "wkc")
        b1own_slice(1)
        wv_c = wchunks(wv_p, wv_d, "wvc")

        # ---- pair AllGather of the Q halves (overlaps B2/B3) -----------
        # the gpsimd FIFO holds only [bvr, trigger, readbacks], so the
        # trigger dispatches the moment the shard DMAs land (~45us).
        nc.gpsimd.collective_compute(
            "AllGather",
            mybir.AluOpType.bypass,
            replica_groups=[[0, 1], [2, 3], [4, 5], [6, 7]],
            ins=[q_shard.opt()],
            outs=[q_full.opt()],
        )
        # readback: slot r holds GLOBAL tokens r*1024:(r+1)*1024; slot 0
        # first (phase C consumes query blocks in global order)
        qall = q_p.tile([P, EC, S], BF16)
        for r in range(2):
            nc.gpsimd.dma_start(
                qall[:, :, r * SK:(r + 1) * SK], q_full[r, :, :, :])

        # ---- Phase B2: K^T (own keys) resident bf16 --------------------
        kt = [kt_p.tile([P, SK], BF16, name=f"kt{ec}", tag="kt")
              for ec in range(EC)]
        for ec in range(EC):
            for kt_i in range(KSL):
                ps = psB.tile([P, NT], F32)
                for dc in range(DC):
                    nc.tensor.matmul(
                        ps[:],
                        (wk_c[dc][:, ec * P:(ec + 1) * P]),
                        (xtall[:, dc, kt_i * NT:(kt_i + 1) * NT]),
                        start=(dc == 0), stop=(dc == DC - 1),
                    )
                nc.scalar.activation(
                    kt[ec][:, kt_i * NT:(kt_i + 1) * NT], ps[:],
                    mybir.ActivationFunctionType.Identity,
                    bias=bkt[:, ec:ec + 1],
                )

        # bv broadcast row -> [128, D] f32 (rank-1 ones x bv matmul)
        bv_rep = const_p.tile([P, D], F32)
        for et in range(D // NT):
            ps = psB.tile([P, NT], F32)
            nc.tensor.matmul(
                ps[:],
                (ones_row[0:1, :]),
                (bvr[0:1, et * NT:(et + 1) * NT]),
                start=True, stop=True,
            )
            nc.scalar.copy(bv_rep[:, et * NT:(et + 1) * NT], ps[:])

        # ---- Phase B3: V natural [k, e] (own keys) resident bf16 -------
        v = [v_p.tile([P, D], BF16, name=f"v{kc}", tag="v")
             for kc in range(KC)]
        for et in range(D // NT):
            for kc in range(KC):
                ps = psB.tile([P, NT], F32)
                for dc in range(DC):
                    nc.tensor.matmul(
                        ps[:],
                        (xtall[:, dc, kc * P:(kc + 1) * P]),
                        (wv_c[dc][:, et * NT:(et + 1) * NT]),
                        start=(dc == 0), stop=(dc == DC - 1),
                    )
                nc.vector.tensor_add(
                    v[kc][:, et * NT:(et + 1) * NT], ps[:],
                    bv_rep[:, et * NT:(et + 1) * NT])

        # ---- Phase C: attention, transposed scores ---------------------
        for qq in range(S // QT):
            # S^T[k, q] per key chunk; exp writes P^T (bf16) straight to SBUF
            ptt = [pt_p.tile([P, QT], BF16, tag="ptp", name=f"ptt{kc}")
                   for kc in range(KC)]
            for kc in range(KC):
                ps = psB.tile([P, QT], F32)
                for ec in range(EC):
                    nc.tensor.matmul(
                        ps[:],
                        (kt[ec][:, kc * P:(kc + 1) * P]),
                        (qall[:, ec, qq * QT:(qq + 1) * QT]),
                        start=(ec == 0), stop=(ec == EC - 1),
                    )
                nc.scalar.activation(
                    ptt[kc][:], ps[:],
                    mybir.ActivationFunctionType.Exp,
                    scale=SCALE,
                )

            # O~ = P^T.T @ V, per 128-query chunk. Paces slower than the
            # ACT exp stream, absorbing its latency.
            for qc in range(QT // P):
                o_sb = io_p.tile([P, D], F32, name="osb", tag="io")
                for et in range(D // NT):
                    ps = psO.tile([P, NT], F32, name="pso")
                    for kc in range(KC):
                        nc.tensor.matmul(
                            ps[:],
                            (ptt[kc][:, qc * P:(qc + 1) * P]),
                            (v[kc][:, et * NT:(et + 1) * NT]),
                            start=(kc == 0), stop=(kc == KC - 1),
                        )
                    nc.vector.tensor_copy(
                        o_sb[:, et * NT:(et + 1) * NT], ps[:])
                row0 = qq * QT + qc * P
                nc.sync.dma_start(o_d[row0:row0 + P, :], o_sb[:])

            # partial row-sums: DVE add-tree over the 8 P^T tiles, then a
            # single ones-vector matmul (f32r, 1 col/cycle).
            def _radd(a, b):
                t = rsum_p.tile([P, QT], F32R, name="racc", tag="racc")
                nc.vector.tensor_add(t[:], a, b)
                return t
            s01 = _radd(ptt[0][:], ptt[1][:])
            s23 = _radd(ptt[2][:], ptt[3][:])
            s45 = _radd(ptt[4][:], ptt[5][:])
            s67 = _radd(ptt[6][:], ptt[7][:])
            s03 = _radd(s01[:], s23[:])
            s47 = _radd(s45[:], s67[:])
            tot = _radd(s03[:], s47[:])
            ps_rs = psA.tile([1, QT], F32, name="ps_rs")
            nc.tensor.matmul(
                ps_rs[:], (ones[:, 0:1]), (tot[:]), start=True, stop=True)
            rs_sb = st_p.tile([1, QT], F32, name="rs_sb", tag="rs")
            nc.vector.tensor_copy(rs_sb[:], ps_rs[:])
            nc.sync.dma_start(
                rs_d[qq * QT:(qq + 1) * QT].rearrange("(o q) -> o q", o=1),
                rs_sb[:])

    nc.compile()
    return nc


_CACHE: dict = {}


def _get_program() -> bass.Bass:
    if "nc" not in _CACHE:
        _CACHE["nc"] = build_program()
    return _CACHE["nc"]


def kernel(x, Wq, bq, Wk, bk, Wv, bv, _trace=False, _trace_kwargs=None):
    nc = _get_program()
    x = np.asarray(x, dtype=np.float32)
    shared = {
        "Wq": np.ascontiguousarray(np.asarray(Wq, np.float32).astype(NP_BF16)),
        "bq": np.ascontiguousarray(np.asarray(bq, np.float32)),
        "Wk": np.ascontiguousarray(np.asarray(Wk, np.float32).astype(NP_BF16)),
        "bk": np.ascontiguousarray(np.asarray(bk, np.float32)),
        "Wv": np.ascontiguousarray(np.asarray(Wv, np.float32).astype(NP_BF16)),
        "bv": np.ascontiguousarray(np.asarray(bv, np.float32)),
    }
    in_maps = []
    for c in range(8):
        b, h = divmod(c, 2)
        xkv = x[b].T[:, h * SK:(h + 1) * SK]  # my key-half columns of x^T
        in_maps.append(
            {"xTkv": np.ascontiguousarray(xkv.astype(NP_BF16)), **shared})

    res = run_bass_kernel_spmd(
        nc, in_maps, list(range(8)),
        trace=_trace, **(_trace_kwargs or {}),
    )
    out = np.empty((4, S, D), dtype=np.float32)
    for b in range(4):
        o0 = res.results[2 * b]["o_raw"].astype(np.float64)
        r0 = res.results[2 * b]["rs_raw"].astype(np.float64)
        o1 = res.results[2 * b + 1]["o_raw"].astype(np.float64)
        r1 = res.results[2 * b + 1]["rs_raw"].astype(np.float64)
        out[b] = ((o0 + o1) / (r0 + r1)[:, None]).astype(np.float32)
    if _trace:
        return out, res
    return out
